# revision 41
# baseline (speedup 1.0000x reference)
"""Causal multi-head attention on 8 Trainium2 NeuronCores.

Full module: x:(2,2048,1024) f32, 16 heads, head_dim 64, causal softmax
(scaled by 1/sqrt(1024)), out = attn(x) @ Wo + bo.

Sharding: core c handles batch b = c // 4 and head group g = c % 4
(4 heads, i.e. 256 columns of Wq/Wk/Wv and 256 rows of Wo). Every core
runs the same program (SPMD); the host sums the 4 per-group partial
output projections per batch and adds the bias.

Per-core kernel layout strategy (all matmuls contract over the SBUF
partition dim; "T" tensors are stored feature-major so no transposes are
needed mid-attention):
  xT   [f=1024, t=2048]  bf16 (+ an fp8 copy x8T), host-transposed, DMA
  QT/KT[d=256,  t=2048]  = fp8 Wq/Wk as lhsT, fp8 xT as rhs, DoubleRow
                          (2 feature chunks per matmul); weights are
                          host-prescaled by 32 past the e4m3 denormal
                          floor, the 1024x is folded into the exp scale
  V    [t=2048, d=256]   = bf16 xT as lhsT, Wv as rhs; stored
                          [128,16,4,65] with a ones column per head
                          (fused softmax sums)
  S^T  [k=128, q<=512]   = KT-slice as lhsT, QT-slice as rhs, two heads
                          on disjoint partition halves; on diagonal
                          chunks S and exp cover only the live columns
  P^T  = exp(S^T/32768) via ScalarE; causal triangle masked by a
         128-wide affine_select on the boundary sub-block only
  ctxT [d=64|sums, q]    = [V|1] as lhsT, P^T as rhs, accumulated in a
                          single [65,1024] PSUM tile per head pair
  norm: one staging copy pv->SBUF (frees the pv banks for the next
        pair), one wide reciprocal of both sums rows, f32r PE
        outer-products into the mm banks, one multiply per head
  out  [t, 1024] bf16    = ctxT as lhsT, Wo as rhs  (partial; the host
                          upcasts, sums the 4 groups and adds the bias)

Scheduling: the TileScheduler dispatches ready work per engine, so
emission order mostly sets priority — but PSUM pool slots rotate FIFO
per tag, so rotation order must follow dependency order.  outproj(qb)
is emitted right after att(qb) (its matmuls fill the next attention's
ACT-bound gaps), the body-boundary proj block draws from the "ps_s"
tag so consecutive repetition bodies overlap, weights load once and
stay resident, and the kernel tail alternates DVE/ScalarE evictions
and the two HWDGE rings to drain 2-wide.
"""

import os

import numpy as np

N = 2048        # tokens per batch
D = 1024        # model dim
HG = 4          # heads per core
HD = 64         # head dim
DG = HG * HD    # 256, feature columns per core
SCALE = 1.0 / 32.0  # 1/sqrt(D); note module scales by sqrt(d_out), not head_dim
NCORES = 8

# tuning knobs (env-overridable for experiments)
PT_BUFS = int(os.environ.get("PT_BUFS", "8"))
# fp8(e4m3) Q/K projections with DoubleRow (2 contraction rows/cycle).
# Wq/Wk are prescaled by 32 on the host so their entries clear the e4m3
# denormal floor; the extra 32*32 factor is folded into the exp scale.
QK_FP8 = os.environ.get("QK_FP8", "1") != "0"
WSCALE = 32.0

_CACHE = {}


def _build_nc(repeat=1):
    from contextlib import ExitStack

    import concourse.mybir as mybir
    import concourse.tile as tile
    from concourse import bacc

    FP32 = mybir.dt.float32
    F32R = mybir.dt.float32r
    BF16 = mybir.dt.bfloat16
    FP8 = mybir.dt.float8e4
    EXP = mybir.ActivationFunctionType.Exp
    COPY = mybir.ActivationFunctionType.Copy
    DROW = mybir.MatmulPerfMode.DoubleRow
    exp_scale = SCALE / (WSCALE * WSCALE) if QK_FP8 else SCALE

    NT = N // 128   # 16 token chunks
    NF = D // 128   # 8 feature chunks
    NQ = N // 512   # 4 query blocks

    nc = bacc.Bacc("TRN2", target_bir_lowering=False, debug=False)

    x_d = nc.dram_tensor("x", [D, N], BF16, kind="ExternalInput").ap()
    qk_dt = FP8 if QK_FP8 else BF16
    if QK_FP8:
        x8_d = nc.dram_tensor("x8", [D, N], FP8, kind="ExternalInput").ap()
    wq_d = nc.dram_tensor("wq", [D, DG], qk_dt, kind="ExternalInput").ap()
    wk_d = nc.dram_tensor("wk", [D, DG], qk_dt, kind="ExternalInput").ap()
    wv_d = nc.dram_tensor("wv", [D, DG], BF16, kind="ExternalInput").ap()
    wo_d = nc.dram_tensor("wo", [DG, D], BF16, kind="ExternalInput").ap()
    out_d = nc.dram_tensor("out", [N, D], BF16, kind="ExternalOutput").ap()

    with tile.TileContext(nc) as tc, ExitStack() as ctx:
        persist = ctx.enter_context(tc.tile_pool(name="persist", bufs=1))
        ptpool = ctx.enter_context(tc.tile_pool(name="ptpool", bufs=PT_BUFS))
        smpool = ctx.enter_context(tc.tile_pool(name="smpool", bufs=4))
        opool = ctx.enter_context(tc.tile_pool(name="opool", bufs=6))
        cbpool = ctx.enter_context(tc.tile_pool(name="cbpool", bufs=4))
        # PSUM budget (8 banks): proj/outproj/bc tag "ps" [128,512]x2 =
        # 2 banks; attention S tag "ps_s" [128,1024]x2 = 4 banks
        # (independent rotation so S runs ahead of exp); pv 2 banks.
        mmpsum = ctx.enter_context(tc.tile_pool(name="mmpsum", bufs=2, space="PSUM"))
        spsum = mmpsum
        pvpsum = ctx.enter_context(tc.tile_pool(name="pvpsum", bufs=1, space="PSUM"))

        # ---- persistent tensors ----
        xT = persist.tile([128, NF, N], BF16, name="xT")          # 32 KB/p
        qt = persist.tile([128, 2, N], BF16, name="qt")           # 8 KB/p
        kt = persist.tile([128, 2, N], BF16, name="kt")           # 8 KB/p
        vt = persist.tile([128, NT, HG, HD + 1], BF16, name="vt")  # ~8 KB/p
        ctxT = persist.tile([128, 2, N], BF16, name="ctxT")       # 8 KB/p
        wq_bf = persist.tile([128, NF, DG], qk_dt, name="wq_bf")  # <=4 KB/p
        wk_bf = persist.tile([128, NF, DG], qk_dt, name="wk_bf")
        wv_bf = persist.tile([128, NF, DG], BF16, name="wv_bf")
        if QK_FP8:
            x8T = persist.tile([128, NF, N], FP8, name="x8T")     # 16 KB/p
        wo_bf = persist.tile([128, 2, D], BF16, name="wo_bf")     # 4 KB/p
        ones128 = persist.tile([128, HD], F32R, name="ones128")

        # walrus requires f32r operands produced by a rounding op
        ones_f32 = persist.tile([128, HD], FP32, name="ones_f32")
        nc.gpsimd.memset(ones_f32[:, :], 1.0)
        nc.vector.tensor_copy(ones128[:, :], ones_f32[:, :])
        nc.gpsimd.memset(vt[:, :, :, HD], 1.0)  # softmax-sum ones columns

        def emit_weights():
            # ---- weights arrive bf16 from the host. They ride the ACT
            # HWDGE ring so they don't queue behind x on the SP ring:
            # proj0's first matmul only needs wq + the first x quarter.
            for w_dram, w_bf in ((wq_d, wq_bf), (wk_d, wk_bf), (wv_d, wv_bf)):
                nc.scalar.dma_start(out=w_bf[:, :, :],
                                    in_=w_dram.rearrange("(c p) d -> p c d", p=128))
            nc.scalar.dma_start(out=wo_bf[:, :, :],
                                in_=wo_d.rearrange("(c p) d -> p c d", p=128))

        def emit_xt_block(ib):
            """DMA the host-transposed x slabs for one 512-token range.
            The first block lands in quarters so proj0 starts sooner."""
            src = x_d.rearrange("(c p) t -> p c t", p=128)
            src8 = x8_d.rearrange("(c p) t -> p c t", p=128) if QK_FP8 else None
            if ib == 0:
                if QK_FP8:
                    # fp8 slab gates the first q/k matmuls: quarters first
                    for h in range(4):
                        nc.sync.dma_start(
                            out=x8T[:, 2 * h:2 * (h + 1), 0:512],
                            in_=src8[:, 2 * h:2 * (h + 1), 0:512])
                    for h in range(2):
                        nc.sync.dma_start(
                            out=xT[:, 4 * h:4 * (h + 1), 0:512],
                            in_=src[:, 4 * h:4 * (h + 1), 0:512])
                else:
                    for h in range(4):
                        nc.sync.dma_start(
                            out=xT[:, 2 * h:2 * (h + 1), 0:512],
                            in_=src[:, 2 * h:2 * (h + 1), 0:512])
            else:
                if QK_FP8:
                    nc.sync.dma_start(
                        out=x8T[:, :, 512 * ib:512 * (ib + 1)],
                        in_=src8[:, :, 512 * ib:512 * (ib + 1)])
                nc.sync.dma_start(
                    out=xT[:, :, 512 * ib:512 * (ib + 1)],
                    in_=src[:, :, 512 * ib:512 * (ib + 1)])

        def proj_units(ib):
            """Thunks projecting one 512-token range of xT into QT/KT/V,
            one PSUM-group per thunk so they can be paced as PE filler
            inside ACT-bound attention windows.

            The body-boundary block (ib==0) draws its PSUM from the
            "ps_s" tag: the "ps" tag's previous users are the prior
            body's tail out-proj tiles (released at its very end), while
            "ps_s" (the S tiles) frees once the prior attention finishes
            — so the next body's projections overlap the prior tail."""
            tag = "ps_s" if ib == 0 else "ps"
            tb = ib
            tsl = slice(512 * tb, 512 * (tb + 1))
            units = []

            def qk_unit(w_bf, dst, dh):
                def emit():
                    dsl = slice(128 * dh, 128 * (dh + 1))
                    ps = mmpsum.tile([128, 512], FP32, name="ps", tag=tag)
                    if QK_FP8:
                        # DoubleRow: 2 feature chunks (256 contraction
                        # rows) per matmul via the [128, 2, n] APs
                        for fc in range(NF // 2):
                            nc.tensor.matmul(
                                ps[:, :],
                                lhsT=w_bf[:, 2 * fc:2 * fc + 2, dsl],
                                rhs=x8T[:, 2 * fc:2 * fc + 2, tsl],
                                start=(fc == 0), stop=(fc == NF // 2 - 1),
                                perf_mode=DROW,
                            )
                    else:
                        for fc in range(NF):
                            nc.tensor.matmul(
                                ps[:, :],
                                lhsT=w_bf[:, fc, dsl],
                                rhs=xT[:, fc, tsl],
                                start=(fc == 0), stop=(fc == NF - 1),
                            )
                    nc.vector.tensor_copy(dst[:, dh, tsl], ps[:, :])
                return emit

            def v_unit(tcc):
                def emit():
                    ps = mmpsum.tile([128, 512], FP32, name="ps", tag=tag)
                    for fc in range(NF):
                        nc.tensor.matmul(
                            ps[:, 0:DG],
                            lhsT=xT[:, fc, 128 * tcc:128 * (tcc + 1)],
                            rhs=wv_bf[:, fc, :],
                            start=(fc == 0), stop=(fc == NF - 1),
                        )
                    nc.vector.tensor_copy(
                        vt[:, tcc, :, 0:HD],
                        ps[:, 0:DG].rearrange("p (h e) -> p h e", h=HG))
                return emit

            for w_bf, dst in ((wq_bf, qt), (wk_bf, kt)):
                for dh in range(2):
                    units.append(qk_unit(w_bf, dst, dh))
            for tcc in range(4 * ib, 4 * ib + 4):
                units.append(v_unit(tcc))
            return units

        def emit_attention(qb, fillers=()):
            """Attention for one 512-wide query block, both head pairs.

            `fillers` are PE-work thunks (out-proj items / next proj
            groups) paced one-per-chunk-iteration through the window:
            the per-chunk PE gap in this ACT-bound loop is shorter than
            one matmul, so bunching fillers at the window start (their
            natural priority order) leaves the back half of the window
            with an idle PE. Leftovers drain after the pair loop."""
            fillers = list(fillers)
            nit = 0
            nit_total = 4 * (qb + 1) * 2
            nfill = 0
            nkc = 4 * (qb + 1)             # causal: k chunks 0..4qb+3
            qsl = slice(512 * qb, 512 * (qb + 1))
            for p in range(2):             # head pair (heads 2p, 2p+1)
                # one [65,1024] tile (2 banks): head A in columns 0-511,
                # head B in 512-1023, so the whole epilogue runs as
                # single wide ops instead of two per step
                pv = pvpsum.tile([HD + 1, 1024], FP32, name="pv", tag="pv")
                for kc in range(nkc):
                    ksl = slice(128 * kc, 128 * (kc + 1))
                    # columns q_local < 128*m are entirely above the
                    # causal diagonal for this k chunk: skip them.
                    m = max(0, kc - 4 * qb)
                    q0 = 128 * m
                    ps_s = spsum.tile([128, 1024], FP32, name="ps_s",
                                      tag="ps_s", bufs=2)
                    # head A on partitions 0-63, head B on 64-127
                    for i in range(2):
                        lo = 64 * i
                        nc.tensor.matmul(
                            ps_s[:, 512 * i + q0:512 * (i + 1)],
                            lhsT=kt[lo:lo + 64, p, ksl],
                            rhs=qt[lo:lo + 64, p,
                                   512 * qb + q0:512 * (qb + 1)],
                            start=True, stop=True,
                        )
                    pt = ptpool.tile([128, 1024], BF16, name="pt")
                    if q0 == 0:
                        nc.scalar.activation(pt[:, :], ps_s[:, :], EXP,
                                             scale=exp_scale)
                    else:
                        # one strided-AP exp covering both heads' live
                        # spans: halves the per-chunk ACT overhead on
                        # the diagonal, straight off the binding floor
                        pt_v = pt[:, :].rearrange("p (h q) -> p h q", h=2)
                        ps_v = ps_s[:, :].rearrange("p (h q) -> p h q", h=2)
                        nc.scalar.activation(pt_v[:, :, q0:512],
                                             ps_v[:, :, q0:512],
                                             EXP, scale=exp_scale)
                    if kc >= 4 * qb:
                        # diagonal: zero q < k on the 128-wide boundary
                        # sub-block; columns beyond it always keep
                        for i in range(2):
                            sl = slice(512 * i + q0, 512 * i + q0 + 128)
                            nc.gpsimd.affine_select(
                                out=pt[:, sl], in_=pt[:, sl],
                                compare_op=mybir.AluOpType.is_ge,
                                fill=0.0,
                                base=0,
                                pattern=[[1, 128]],
                                channel_multiplier=-1,
                            )
                    st = (kc == 0)
                    sp = (kc == nkc - 1)
                    nc.tensor.matmul(
                        pv[:, q0:512], lhsT=vt[:, kc, 2 * p, :],
                        rhs=pt[:, q0:512], start=st, stop=sp,
                    )
                    nc.tensor.matmul(
                        pv[:, 512 + q0:1024], lhsT=vt[:, kc, 2 * p + 1, :],
                        rhs=pt[:, 512 + q0:1024], start=st, stop=sp,
                    )
                    nit += 1
                    while (nfill < len(fillers)
                           and nfill < nit * len(fillers) // nit_total):
                        fillers[nfill]()
                        nfill += 1
                # epilogue: ONE wide staging copy of the whole pv tile
                # (values + sums rows) releases the pv banks for the next
                # pair as early as possible; the reciprocal then runs from
                # SBUF, the PE outer-product broadcasts land in the idle
                # mm banks, and each head gets one multiply (SBUF x PSUM).
                # The kernel-tail pair stages via the by-then-idle ScalarE.
                tail = (qb == NQ - 1 and p == 1)
                st2 = cbpool.tile([HD + 1, 1024], FP32, name="st2", tag="st")
                if tail:
                    nc.scalar.activation(st2[:, :], pv[:, :], COPY)
                else:
                    nc.vector.tensor_copy(st2[:, :], pv[:, :])
                rec = smpool.tile([HD + 1, 1024], F32R, name="rec")
                with nc.allow_low_precision(reason="f32r softmax recip"):
                    nc.vector.reciprocal(rec[HD:HD + 1, :],
                                         st2[HD:HD + 1, :])
                bc_a = mmpsum.tile([HD, 512], FP32, name="bc_a", tag="ps")
                bc_b = mmpsum.tile([HD, 512], FP32, name="bc_b", tag="ps")
                ones_ap = ones128[HD:HD + 1, :]
                nc.tensor.matmul(bc_b[:, :], lhsT=ones_ap,
                                 rhs=rec[HD:HD + 1, 512:1024],
                                 start=True, stop=True)
                nc.tensor.matmul(bc_a[:, :], lhsT=ones_ap,
                                 rhs=rec[HD:HD + 1, 0:512],
                                 start=True, stop=True)
                # head B first: its result still has a DMA hop to make
                # partitions 64-127, which overlaps head A's multiply
                cb = cbpool.tile([HD, 512], BF16, name="cb", tag="cb")
                nc.vector.tensor_mul(cb[:, :], st2[0:HD, 512:1024],
                                     bc_b[:, :])
                nc.sync.dma_start(out=ctxT[HD:128, p, qsl], in_=cb[:, :])
                nc.vector.tensor_mul(ctxT[0:HD, p, qsl], st2[0:HD, 0:512],
                                     bc_a[:, :])
            while nfill < len(fillers):
                fillers[nfill]()
                nfill += 1

        def outproj_units(qb, tail):
            """Thunks for the output projection of one 512-token range
            (partial over heads), evicted to bf16. During attention
            windows evictions go to DVE and DMAs ride the SP HWDGE ring;
            the kernel-tail block alternates DVE/ScalarE evictions and
            SP/ACT DMA rings (exps are done by then) to drain 2-wide."""
            def item(tb, nh):
                def emit():
                    tsl = slice(128 * tb, 128 * (tb + 1))
                    ps_o = mmpsum.tile([128, 512], FP32, name="ps", tag="ps")
                    for hc in range(2):
                        nc.tensor.matmul(
                            ps_o[:, :],
                            lhsT=ctxT[:, hc, tsl],
                            rhs=wo_bf[:, hc, 512 * nh:512 * (nh + 1)],
                            start=(hc == 0), stop=(hc == 1),
                        )
                    o_sb = opool.tile([128, 512], BF16, name="o_sb")
                    if tail and nh == 1:
                        nc.scalar.activation(o_sb[:, :], ps_o[:, :], COPY)
                    else:
                        nc.vector.tensor_copy(o_sb[:, :], ps_o[:, :])
                    dma_eng = nc.scalar if (tail and nh == 1) else nc.sync
                    dma_eng.dma_start(
                        out=out_d[tsl, 512 * nh:512 * (nh + 1)],
                        in_=o_sb[:, :])
                return emit
            return [item(tb, nh)
                    for tb in range(4 * qb, 4 * qb + 4) for nh in range(2)]

        def run_all(units):
            for u in units:
                u()

        def emit_body():
            emit_xt_block(0)
            run_all(proj_units(0))
            emit_xt_block(1)
            run_all(proj_units(1))
            emit_attention(0, [lambda: emit_xt_block(2)] + proj_units(2))
            emit_attention(1, outproj_units(0, tail=False)
                           + [lambda: emit_xt_block(3)] + proj_units(3))
            emit_attention(2, outproj_units(1, tail=False))
            emit_attention(3, outproj_units(2, tail=False))
            run_all(outproj_units(3, tail=True))

        # weights are loaded once and stay resident across bodies (the
        # repetition bodies model steady-state serving)
        emit_weights()
        for _rep in range(repeat):
            emit_body()

    nc.compile()
    return nc


def _get_nc(repeat=1):
    key = ("nc", repeat)
    if key not in _CACHE:
        _CACHE[key] = _build_nc(repeat)
    return _CACHE[key]


def _make_in_maps(x, Wq, Wk, Wv, Wo):
    import ml_dtypes
    bf = ml_dtypes.bfloat16
    f8 = ml_dtypes.float8_e4m3
    qk_dt = f8 if QK_FP8 else bf
    qk_s = WSCALE if QK_FP8 else 1.0
    in_maps = []
    xbT = [np.ascontiguousarray(x[b].T) for b in range(2)]
    for c in range(NCORES):
        b, g = divmod(c, 4)
        cs = slice(DG * g, DG * (g + 1))
        m = {
            "x": xbT[b].astype(bf),
            "wq": np.ascontiguousarray(qk_s * Wq[:, cs]).astype(qk_dt),
            "wk": np.ascontiguousarray(qk_s * Wk[:, cs]).astype(qk_dt),
            "wv": np.ascontiguousarray(Wv[:, cs]).astype(bf),
            "wo": np.ascontiguousarray(Wo[cs, :]).astype(bf),
        }
        if QK_FP8:
            m["x8"] = xbT[b].astype(f8)
        in_maps.append(m)
    return in_maps


def _gather(results, bo):
    out = np.empty((2, N, D), dtype=np.float32)
    for b in range(2):
        acc = results[4 * b]["out"].astype(np.float32)
        for g in range(1, 4):
            acc = acc + results[4 * b + g]["out"].astype(np.float32)
        out[b] = acc + bo[None, :].astype(np.float32)
    return out


def run_spmd(x, Wq, Wk, Wv, Wo, bo, **spmd_kwargs):
    """Run the 8-core kernel; returns (full_output, BassKernelResults)."""
    from concourse.bass_utils import run_bass_kernel_spmd

    nc = _get_nc()
    in_maps = _make_in_maps(
        np.asarray(x), np.asarray(Wq), np.asarray(Wk), np.asarray(Wv),
        np.asarray(Wo))
    res = run_bass_kernel_spmd(nc, in_maps, core_ids=list(range(NCORES)),
                               **spmd_kwargs)
    return _gather(res.results, np.asarray(bo)), res


def kernel(x, Wq, Wk, Wv, Wo, bo):
    out, _ = run_spmd(x, Wq, Wk, Wv, Wo, bo)
    return out
